# revision 27
# baseline (speedup 1.0000x reference)
"""Masked grouped Conv1D (CustomMaskedConv1D) Trainium2 Bass kernel.

Problem (reference semantics):
  inputs    [B=4, L=4096, C=1024] f32
  positions [B=4, L=4096] i32 (sorted)
  kernel    [G=16, OPG=64, IPG=64, K=5] f32
  out[b,l,g,o] = sum_k mask[b,l,k] * sum_i x_pad[b, l+k-2, g*64+i] * W[g,o,i,k]
  mask[b,l,k] = (pos_pad[b, l+k-2] == pos[b,l] + k - 2)

Strategy: data-parallel over (batch x half-sequence) -> 8 shards of 2048 rows
(+2 halo rows each side). Host does lossless layout transforms only
(slicing, zero-pad, transpose, compact weight repack); all dtype casts and
all arithmetic run on device.

Device pipeline per core (DMA-roofline oriented):
  - x + positions read on the SP HWDGE ring (x0 first); weights read as 5
    per-tap compact chunks on the ACT HWDGE ring (both rings feed the same
    16 SDMA engines, so reads/weights/writes flow concurrently)
  - weights shipped compact f32 [128, 40*64] (no block-diag zeros over
    HBM); per-tap strided ACT casts place the diagonal 64x64 blocks into a
    DVE-zeroed bf16 [128, 5120] block-diag lhsT table, in PE consumption
    order (k2 first) so the first matmuls unblock early
  - masks via e-trick: e[j] = pos[j] - j (iota on gpsimd, subtract on DVE,
    all i16 -> DVE 2x mode), then msk_k = is_equal(e[2:], e[4-k:]) as bf16,
    each mask interleaved right before its first production
  - per chunk cc (128 channels = 2 groups): ACT casts x f32->bf16,
    DVE produces ym_k = x * msk_k (bf16, 2x mode), PE runs 20 block-diag
    matmuls (5 taps x 4 n-blocks of 512) accumulating into a [128, 2048]
    psum tile (4 banks), ACT copies psum->SBUF, ACT-ring DMA writes out^T;
    issue order software-pipelines cast(cc+1) ahead of copy(cc)
  - 28 dummy matmuls at kernel start keep the PE HAM clock-gate warm; the
    last chunk drains per-psum-bank (ACT/DVE alternating, sync-ring writes)
    to shorten the kernel tail
Host gathers/transposes shards into [4, 4096, 16, 64].
"""

import os

import numpy as np

import concourse.bass as bass
import concourse.mybir as mybir
import concourse.tile as tile
from concourse import bacc
from concourse.bass_utils import run_bass_kernel_spmd

B, L, C = 4, 4096, 1024
G, OPG, IPG, K = 16, 64, 64, 5
HALO = K // 2  # 2

NCORES = 8
NR = (B * L) // NCORES  # 2048 output rows per core
NP = NR + 2 * HALO  # 2052 padded rows per core
NCC = C // 128  # 8 channel chunks == group pairs
NNB = NR // 512  # 4 n-blocks of 512
NBLK = K * NCC  # 40 weight blocks
TAPS_OFF = (0, 1, 3, 4)  # off-center taps (center tap k=2 has mask==1)
N_WARM = 28  # PE HAM warm-up matmuls

# cache the compiled Bass program + results of the last run
_NC = None
LAST_RESULTS = None


def _build():
    nc = bacc.Bacc(
        "TRN2", target_bir_lowering=False, debug=False, num_devices=NCORES
    )
    bf16 = mybir.dt.bfloat16
    f32 = mybir.dt.float32
    i16 = mybir.dt.int16

    xt_dram = nc.dram_tensor("xt", [C, NP], f32, kind="ExternalInput")
    # positions replicated across 128 partitions on host (layout transform);
    # col j corresponds to padded row m = j - 2, cols [2, NP+2) are real
    ps_dram = nc.dram_tensor("ps", [128, NP + 4], i16, kind="ExternalInput")
    # compact weights [128 ch-in-pair, NBLK*64]: rows 0:64 group (2cc),
    # rows 64:128 group (2cc+1); block j = k*NCC+cc holds W[g,:,:,k].T
    w_dram = nc.dram_tensor("w", [128, NBLK * 64], f32, kind="ExternalInput")
    out_dram = nc.dram_tensor("out", [C, NR], f32, kind="ExternalOutput")

    with tile.TileContext(nc) as tc:
        with (
            tc.tile_pool(name="persist", bufs=1) as pers,
            tc.tile_pool(name="setup", bufs=1) as setup,
            tc.tile_pool(name="stage", bufs=5) as stage,
            tc.tile_pool(name="xbf", bufs=3) as xbf,
            tc.tile_pool(name="ym", bufs=2) as ymp,
            tc.tile_pool(name="osb", bufs=3) as osb,
            tc.tile_pool(name="psum", bufs=2, space="PSUM") as pp,
        ):
            # ---- t0 work on idle engines (no input dependencies) ----
            # iota first: it gates the whole mask chain (e -> msk -> ym)
            iot = pers.tile([128, NP + 4], i16, tag="iota")
            nc.gpsimd.iota(iot[:], pattern=[[1, NP + 4]], base=0,
                           channel_multiplier=0)
            dmy = pers.tile([128, 512], bf16, tag="dmy")
            nc.gpsimd.memset(dmy[:], 0)
            # bf16 block-diag lhsT table: DVE zeros it (DVE is idle until the
            # mask chain starts), ACT casts the diagonal 64x64 blocks in
            w_sb = pers.tile([128, NBLK * 128], bf16, tag="w")
            nc.vector.memset(w_sb[:], 0)

            # ---- reads: SP ring (x0 first: it gates the whole x pipeline) ----
            ps_bc = setup.tile([128, NP + 4], i16, tag="psbc")
            x32s = []
            for cc in range(NCC):
                x32 = stage.tile([128, NP], f32, tag="x32")
                nc.sync.dma_start(x32[:], xt_dram[cc * 128 : (cc + 1) * 128, :])
                x32s.append(x32)
                if cc == 0:
                    nc.sync.dma_start(ps_bc[:], ps_dram[:])

            # ---- weights: 5 per-tap reads on the ACT ring (runs in parallel
            # with the SP-ring reads), dispatched in PE consumption order ----
            w32k = {}
            for k in (2, 0, 1, 3, 4):
                wt = setup.tile([128, NCC * 64], f32, tag=f"w32k{k}")
                nc.scalar.dma_start(wt[:], w_dram[:, k * NCC * 64 : (k + 1) * NCC * 64])
                w32k[k] = wt

            # ---- PE warm-up (keeps HAM clock-gate at 8/8 for real matmuls) ----
            pdum = pp.tile([128, 512], f32, tag="acc", name="pdum")
            for i in range(N_WARM):
                nc.tensor.matmul(pdum[:], dmy[:, 0:128], dmy[:], start=True,
                                 stop=True)

            # ---- masks: e[j] = ps[j] - j, msk_k = (e[2+m] == e[4-k+m]) ----
            # (mask compares are interleaved with cc0's productions below)
            e_t = setup.tile([128, NP + 4], i16, tag="e")
            nc.vector.tensor_tensor(out=e_t[:], in0=ps_bc[:], in1=iot[:],
                                    op=mybir.AluOpType.subtract)
            msk = {}

            def make_msk(k):
                m = pers.tile([128, NP], bf16, tag=f"msk{k}")
                nc.vector.tensor_tensor(
                    out=m[:], in0=e_t[:, 2 : NP + 2],
                    in1=e_t[:, 4 - k : NP + 4 - k],
                    op=mybir.AluOpType.is_equal,
                )
                msk[k] = m

            def wcast(k):
                # strided casts place tap k's diagonal blocks: partitions 0:64
                # -> block cols [0:64), partitions 64:128 -> [64:128)
                reg = w_sb[:, k * NCC * 128 : (k + 1) * NCC * 128]
                for lo in (0, 64):
                    src = w32k[k][lo : lo + 64, :].rearrange(
                        "p (b c) -> p b c", c=64)
                    dst = reg[lo : lo + 64, :].rearrange(
                        "p (b c) -> p b c", c=128)[:, :, lo : lo + 64]
                    nc.scalar.copy(dst, src)

            def xcast(cc):
                xt = xbf.tile([128, NP], bf16, tag="xt")
                nc.scalar.copy(xt[:], x32s[cc][:])
                return xt

            def prods(cc, xt, first=False):
                # production: masked shifted copies (bf16 full width => 2x);
                # first chunk interleaves each mask compare before its product
                ym = {}
                for k in TAPS_OFF:
                    if first:
                        make_msk(k)
                    y = ymp.tile([128, NP], bf16, tag=f"ym{k}")
                    nc.vector.tensor_tensor(
                        out=y[:], in0=xt[:], in1=msk[k][:],
                        op=mybir.AluOpType.mult,
                    )
                    ym[k] = y
                return ym

            def mms(cc, ptile, xt, ym, taps):
                # conv taps; global tap order is (2, 0, 1, 3, 4): start the
                # psum group on k=2, stop it on k=4
                for ki in taps:
                    wcol = (ki * NCC + cc) * 128
                    lhsT = w_sb[:, wcol : wcol + 128]
                    for nb in range(NNB):
                        n0 = nb * 512
                        if ki == 2:
                            rhs = xt[:, n0 + 2 : n0 + 2 + 512]
                        else:
                            rhs = ym[ki][:, n0 + ki : n0 + ki + 512]
                        nc.tensor.matmul(
                            ptile[:, n0 : n0 + 512], lhsT, rhs,
                            start=(ki == 2), stop=(ki == 4),
                        )

            # ---- software-pipelined schedule over channel chunks ----
            # prologue: cc0 splits its taps around cast(1)/wcast(3,4) so the
            # ACT stream is cast0, wc2/0/1, cast1, wc3/4, cast2, copy0, ...
            xt0 = xcast(0)
            for k in (2, 0, 1):
                wcast(k)
            ym0 = prods(0, xt0, first=True)
            pt0 = pp.tile([128, NR], f32, tag="acc", name="acc0")
            mms(0, pt0, xt0, ym0, (2, 0, 1))
            xt1 = xcast(1)
            for k in (3, 4):
                wcast(k)
            mms(0, pt0, xt0, ym0, (3, 4))

            prev = (pt0, 0)
            xt = xt1
            for cc in range(1, NCC):
                xt_next = xcast(cc + 1) if cc + 1 < NCC else None
                ym = prods(cc, xt)
                ptile = pp.tile([128, NR], f32, tag="acc", name=f"acc{cc}")
                mms(cc, ptile, xt, ym, (2, 0, 1, 3, 4))
                _drain(nc, osb, out_dram, prev)
                prev = (ptile, cc)
                xt = xt_next

            # last chunk drains per-bank to shorten the kernel tail
            _drain(nc, osb, out_dram, prev, fine=True)

    nc.compile()
    return nc


def _drain(nc, osb, out_dram, prev, fine=False):
    ptile, cc = prev
    o_sb = osb.tile([128, NR], mybir.dt.float32, tag="osb")
    rows = out_dram[cc * 128 : (cc + 1) * 128, :]
    if fine:
        # ACT and DVE alternate bank copies (both idle at the tail); all
        # writes dispatch on the idle sync ring so they overlap the copies
        for nb in range(NNB):
            n0 = nb * 512
            if nb % 2 == 0:
                nc.scalar.copy(o_sb[:, n0 : n0 + 512], ptile[:, n0 : n0 + 512])
            else:
                nc.vector.tensor_copy(o_sb[:, n0 : n0 + 512],
                                      ptile[:, n0 : n0 + 512])
            nc.sync.dma_start(rows[:, n0 : n0 + 512], o_sb[:, n0 : n0 + 512])
    else:
        nc.scalar.copy(o_sb[:], ptile[:])
        nc.scalar.dma_start(rows, o_sb[:])


def _get_nc():
    global _NC
    if _NC is None:
        _NC = _build()
    return _NC


def _shard_inputs(inputs, positions, kernel):
    """Host-side lossless layout transforms: slice+pad shards, transpose x,
    compact weight packing. No dtype-lossy changes (positions < 2**15 fit
    int16 exactly)."""
    in_maps = []
    # compact weights [128, NBLK*64] f32: block j = k*NCC + cc
    w_c = np.zeros((128, NBLK * 64), dtype=np.float32)
    for k in range(K):
        for cc in range(NCC):
            j = k * NCC + cc
            w_c[0:64, j * 64 : (j + 1) * 64] = kernel[2 * cc, :, :, k].T
            w_c[64:128, j * 64 : (j + 1) * 64] = kernel[2 * cc + 1, :, :, k].T

    half = L // 2  # 2048
    for core in range(NCORES):
        b, h = divmod(core, 2)
        l0 = h * half
        xs = np.zeros((NP, C), dtype=np.float32)
        ps = np.full((NP + 4,), -1, dtype=np.int16)
        lo, hi = l0 - HALO, l0 + half + HALO
        src_lo, src_hi = max(lo, 0), min(hi, L)
        dst_lo = src_lo - lo
        xs[dst_lo : dst_lo + (src_hi - src_lo)] = inputs[b, src_lo:src_hi]
        ps[2 + dst_lo : 2 + dst_lo + (src_hi - src_lo)] = positions[b, src_lo:src_hi]
        ps_bc = np.ascontiguousarray(np.broadcast_to(ps, (128, NP + 4)))
        xt = np.ascontiguousarray(xs.T)  # [C, NP]
        in_maps.append({"xt": xt, "ps": ps_bc, "w": w_c})
    return in_maps


def kernel(inputs, positions, kernel):
    global LAST_RESULTS
    inputs = np.asarray(inputs, dtype=np.float32)
    positions = np.asarray(positions, dtype=np.int32)
    kernel = np.asarray(kernel, dtype=np.float32)

    nc = _get_nc()
    in_maps = _shard_inputs(inputs, positions, kernel)
    res = run_bass_kernel_spmd(
        nc,
        in_maps,
        core_ids=list(range(NCORES)),
        trace=bool(os.environ.get("BASS_TRACE")),
    )
    LAST_RESULTS = res

    out = np.empty((B, L, G, OPG), dtype=np.float32)
    half = L // 2
    for core in range(NCORES):
        b, h = divmod(core, 2)
        l0 = h * half
        # device output is out^T [C=1024 (g*64+o), NR]
        ot = res.results[core]["out"]
        out[b, l0 : l0 + half] = ot.T.reshape(half, G, OPG)
    return out


# revision 31
# speedup vs baseline: 1.0191x; 1.0191x over previous
"""Masked grouped Conv1D (CustomMaskedConv1D) Trainium2 Bass kernel.

Problem (reference semantics):
  inputs    [B=4, L=4096, C=1024] f32
  positions [B=4, L=4096] i32 (sorted)
  kernel    [G=16, OPG=64, IPG=64, K=5] f32
  out[b,l,g,o] = sum_k mask[b,l,k] * sum_i x_pad[b, l+k-2, g*64+i] * W[g,o,i,k]
  mask[b,l,k] = (pos_pad[b, l+k-2] == pos[b,l] + k - 2)

Strategy: data-parallel over (batch x half-sequence) -> 8 shards of 2048 rows
(+2 halo rows each side). Host does lossless layout transforms only
(slicing, zero-pad, transpose, compact weight repack); all dtype casts and
all arithmetic run on device.

Device pipeline per core (DMA-roofline oriented):
  - x + positions read on the SP HWDGE ring (x0 first); weights read as 5
    per-tap compact chunks on the ACT HWDGE ring (both rings feed the same
    16 SDMA engines, so reads/weights/writes flow concurrently)
  - weights shipped compact f32 [128, 40*64] (no block-diag zeros over
    HBM); per-tap strided ACT casts place the diagonal 64x64 blocks into a
    DVE-zeroed bf16 [128, 5120] block-diag lhsT table, in PE consumption
    order (k2 first) so the first matmuls unblock early
  - masks via e-trick: e[j] = pos[j] - j (iota on gpsimd, subtract on DVE,
    all i16 -> DVE 2x mode), then msk_k = is_equal(e[2:], e[4-k:]) as bf16,
    each mask interleaved right before its first production
  - per chunk cc (128 channels = 2 groups): ACT casts x f32->bf16,
    DVE produces ym_k = x * msk_k (bf16, 2x mode), PE runs 20 block-diag
    matmuls (5 taps x 4 n-blocks of 512) accumulating into a [128, 2048]
    psum tile (4 banks), ACT copies psum->SBUF, ACT-ring DMA writes out^T;
    issue order software-pipelines cast(cc+1) ahead of copy(cc)
  - 28 dummy matmuls at kernel start keep the PE HAM clock-gate warm; the
    last chunk drains per-psum-bank (ACT/DVE alternating, sync-ring writes)
    to shorten the kernel tail
Host gathers/transposes shards into [4, 4096, 16, 64].
"""

import os

import numpy as np

import concourse.bass as bass
import concourse.mybir as mybir
import concourse.tile as tile
from concourse import bacc
from concourse.bass_utils import run_bass_kernel_spmd

B, L, C = 4, 4096, 1024
G, OPG, IPG, K = 16, 64, 64, 5
HALO = K // 2  # 2

NCORES = 8
NR = (B * L) // NCORES  # 2048 output rows per core
NP = NR + 2 * HALO  # 2052 padded rows per core
NCC = C // 128  # 8 channel chunks == group pairs
NNB = NR // 512  # 4 n-blocks of 512
NBLK = K * NCC  # 40 weight blocks
TAPS_OFF = (0, 1, 3, 4)  # off-center taps (center tap k=2 has mask==1)
N_WARM = 34  # PE HAM warm-up matmuls

# cache the compiled Bass program + results of the last run
_NC = None
LAST_RESULTS = None


def _build():
    nc = bacc.Bacc(
        "TRN2", target_bir_lowering=False, debug=False, num_devices=NCORES
    )
    bf16 = mybir.dt.bfloat16
    f32 = mybir.dt.float32
    i16 = mybir.dt.int16

    xt_dram = nc.dram_tensor("xt", [C, NP], f32, kind="ExternalInput")
    # positions replicated across 128 partitions on host (layout transform);
    # col j corresponds to padded row m = j - 2, cols [2, NP+2) are real
    ps_dram = nc.dram_tensor("ps", [128, NP + 4], i16, kind="ExternalInput")
    # compact weights [128 ch-in-pair, NBLK*64]: rows 0:64 group (2cc),
    # rows 64:128 group (2cc+1); block j = k*NCC+cc holds W[g,:,:,k].T
    w_dram = nc.dram_tensor("w", [128, NBLK * 64], f32, kind="ExternalInput")
    out_dram = nc.dram_tensor("out", [C, NR], f32, kind="ExternalOutput")

    with tile.TileContext(nc) as tc:
        with (
            tc.tile_pool(name="persist", bufs=1) as pers,
            tc.tile_pool(name="setup", bufs=1) as setup,
            tc.tile_pool(name="stage", bufs=5) as stage,
            tc.tile_pool(name="xbf", bufs=3) as xbf,
            tc.tile_pool(name="ym", bufs=2) as ymp,
            tc.tile_pool(name="osb", bufs=3) as osb,
            tc.tile_pool(name="psum", bufs=2, space="PSUM") as pp,
        ):
            # ---- t0 work on idle engines (no input dependencies) ----
            # iota first: it gates the whole mask chain (e -> msk -> ym)
            iot = pers.tile([128, NP + 4], i16, tag="iota")
            nc.gpsimd.iota(iot[:], pattern=[[1, NP + 4]], base=0,
                           channel_multiplier=0)
            dmy = pers.tile([128, 512], bf16, tag="dmy")
            nc.gpsimd.memset(dmy[:], 0)
            # bf16 block-diag lhsT table: DVE zeros it (DVE is idle until the
            # mask chain starts), ACT casts the diagonal 64x64 blocks in
            w_sb = pers.tile([128, NBLK * 128], bf16, tag="w")
            nc.vector.memset(w_sb[:], 0)

            # ---- reads: SP ring (x0 first: it gates the whole x pipeline) ----
            ps_bc = setup.tile([128, NP + 4], i16, tag="psbc")
            x32s = []
            for cc in range(NCC):
                x32 = stage.tile([128, NP], f32, tag="x32")
                nc.sync.dma_start(x32[:], xt_dram[cc * 128 : (cc + 1) * 128, :])
                x32s.append(x32)
                if cc == 0:
                    nc.sync.dma_start(ps_bc[:], ps_dram[:])

            # ---- weights: 5 per-tap reads on the ACT ring (runs in parallel
            # with the SP-ring reads), dispatched in PE consumption order ----
            w32k = {}
            for k in (2, 0, 1, 3, 4):
                wt = setup.tile([128, NCC * 64], f32, tag=f"w32k{k}")
                nc.scalar.dma_start(wt[:], w_dram[:, k * NCC * 64 : (k + 1) * NCC * 64])
                w32k[k] = wt

            # ---- PE warm-up (keeps HAM clock-gate at 8/8 for real matmuls) ----
            pdum = pp.tile([128, 512], f32, tag="acc", name="pdum")
            for i in range(N_WARM):
                nc.tensor.matmul(pdum[:], dmy[:, 0:128], dmy[:], start=True,
                                 stop=True)

            # ---- masks: e[j] = ps[j] - j, msk_k = (e[2+m] == e[4-k+m]) ----
            # (mask compares are interleaved with cc0's productions below)
            e_t = setup.tile([128, NP + 4], i16, tag="e")
            nc.vector.tensor_tensor(out=e_t[:], in0=ps_bc[:], in1=iot[:],
                                    op=mybir.AluOpType.subtract)
            msk = {}

            def make_msk(k):
                m = pers.tile([128, NP], bf16, tag=f"msk{k}")
                nc.vector.tensor_tensor(
                    out=m[:], in0=e_t[:, 2 : NP + 2],
                    in1=e_t[:, 4 - k : NP + 4 - k],
                    op=mybir.AluOpType.is_equal,
                )
                msk[k] = m

            def wcast(k, eng=None):
                # strided casts place tap k's diagonal blocks: partitions 0:64
                # -> block cols [0:64), partitions 64:128 -> [64:128)
                reg = w_sb[:, k * NCC * 128 : (k + 1) * NCC * 128]
                for lo in (0, 64):
                    src = w32k[k][lo : lo + 64, :].rearrange(
                        "p (b c) -> p b c", c=64)
                    dst = reg[lo : lo + 64, :].rearrange(
                        "p (b c) -> p b c", c=128)[:, :, lo : lo + 64]
                    if eng == "dve":
                        nc.vector.tensor_copy(dst, src)
                    else:
                        nc.scalar.copy(dst, src)

            def xcast(cc):
                xt = xbf.tile([128, NP], bf16, tag="xt")
                nc.scalar.copy(xt[:], x32s[cc][:])
                return xt

            def prods(cc, xt, first=False):
                # production: masked shifted copies (bf16 full width => 2x);
                # first chunk interleaves each mask compare before its product
                # and slots the tap-3/4 weight casts onto DVE mid-sequence so
                # PE's k3/k4 matmuls aren't gated by the ACT stream
                ym = {}
                for k in TAPS_OFF:
                    if first:
                        make_msk(k)
                        if k == 3:
                            wcast(3, eng="dve")
                            wcast(4, eng="dve")
                    y = ymp.tile([128, NP], bf16, tag=f"ym{k}")
                    nc.vector.tensor_tensor(
                        out=y[:], in0=xt[:], in1=msk[k][:],
                        op=mybir.AluOpType.mult,
                    )
                    ym[k] = y
                return ym

            def mms(cc, ptile, xt, ym, taps):
                # conv taps; global tap order is (2, 0, 1, 3, 4): start the
                # psum group on k=2, stop it on k=4
                for ki in taps:
                    wcol = (ki * NCC + cc) * 128
                    lhsT = w_sb[:, wcol : wcol + 128]
                    for nb in range(NNB):
                        n0 = nb * 512
                        if ki == 2:
                            rhs = xt[:, n0 + 2 : n0 + 2 + 512]
                        else:
                            rhs = ym[ki][:, n0 + ki : n0 + ki + 512]
                        nc.tensor.matmul(
                            ptile[:, n0 : n0 + 512], lhsT, rhs,
                            start=(ki == 2), stop=(ki == 4),
                        )

            # ---- software-pipelined schedule over channel chunks ----
            # prologue: cc0 splits its taps around cast(1)/wcast(3,4) so the
            # ACT stream is cast0, wc2/0/1, cast1, wc3/4, cast2, copy0, ...
            xt0 = xcast(0)
            for k in (2, 0, 1):
                wcast(k)
            ym0 = prods(0, xt0, first=True)
            pt0 = pp.tile([128, NR], f32, tag="acc", name="acc0")
            mms(0, pt0, xt0, ym0, (2, 0, 1))
            xt1 = xcast(1)
            mms(0, pt0, xt0, ym0, (3, 4))

            prev = (pt0, 0)
            xt = xt1
            for cc in range(1, NCC):
                xt_next = xcast(cc + 1) if cc + 1 < NCC else None
                ym = prods(cc, xt)
                ptile = pp.tile([128, NR], f32, tag="acc", name=f"acc{cc}")
                mms(cc, ptile, xt, ym, (2, 0, 1, 3, 4))
                _drain(nc, osb, out_dram, prev)
                prev = (ptile, cc)
                xt = xt_next

            # last chunk drains per-bank to shorten the kernel tail
            _drain(nc, osb, out_dram, prev, fine=True)

    nc.compile()
    return nc


def _drain(nc, osb, out_dram, prev, fine=False):
    ptile, cc = prev
    o_sb = osb.tile([128, NR], mybir.dt.float32, tag="osb")
    rows = out_dram[cc * 128 : (cc + 1) * 128, :]
    if fine:
        # ACT and DVE alternate bank copies (both idle at the tail); all
        # writes dispatch on the idle sync ring so they overlap the copies
        for nb in range(NNB):
            n0 = nb * 512
            if nb % 2 == 0:
                nc.scalar.copy(o_sb[:, n0 : n0 + 512], ptile[:, n0 : n0 + 512])
            else:
                nc.vector.tensor_copy(o_sb[:, n0 : n0 + 512],
                                      ptile[:, n0 : n0 + 512])
            nc.sync.dma_start(rows[:, n0 : n0 + 512], o_sb[:, n0 : n0 + 512])
    else:
        nc.scalar.copy(o_sb[:], ptile[:])
        nc.scalar.dma_start(rows, o_sb[:])


def _get_nc():
    global _NC
    if _NC is None:
        _NC = _build()
    return _NC


def _shard_inputs(inputs, positions, kernel):
    """Host-side lossless layout transforms: slice+pad shards, transpose x,
    compact weight packing. No dtype-lossy changes (positions < 2**15 fit
    int16 exactly)."""
    in_maps = []
    # compact weights [128, NBLK*64] f32: block j = k*NCC + cc
    w_c = np.zeros((128, NBLK * 64), dtype=np.float32)
    for k in range(K):
        for cc in range(NCC):
            j = k * NCC + cc
            w_c[0:64, j * 64 : (j + 1) * 64] = kernel[2 * cc, :, :, k].T
            w_c[64:128, j * 64 : (j + 1) * 64] = kernel[2 * cc + 1, :, :, k].T

    half = L // 2  # 2048
    for core in range(NCORES):
        b, h = divmod(core, 2)
        l0 = h * half
        xs = np.zeros((NP, C), dtype=np.float32)
        ps = np.full((NP + 4,), -1, dtype=np.int16)
        lo, hi = l0 - HALO, l0 + half + HALO
        src_lo, src_hi = max(lo, 0), min(hi, L)
        dst_lo = src_lo - lo
        xs[dst_lo : dst_lo + (src_hi - src_lo)] = inputs[b, src_lo:src_hi]
        ps[2 + dst_lo : 2 + dst_lo + (src_hi - src_lo)] = positions[b, src_lo:src_hi]
        ps_bc = np.ascontiguousarray(np.broadcast_to(ps, (128, NP + 4)))
        xt = np.ascontiguousarray(xs.T)  # [C, NP]
        in_maps.append({"xt": xt, "ps": ps_bc, "w": w_c})
    return in_maps


def kernel(inputs, positions, kernel):
    global LAST_RESULTS
    inputs = np.asarray(inputs, dtype=np.float32)
    positions = np.asarray(positions, dtype=np.int32)
    kernel = np.asarray(kernel, dtype=np.float32)

    nc = _get_nc()
    in_maps = _shard_inputs(inputs, positions, kernel)
    res = run_bass_kernel_spmd(
        nc,
        in_maps,
        core_ids=list(range(NCORES)),
        trace=bool(os.environ.get("BASS_TRACE")),
    )
    LAST_RESULTS = res

    out = np.empty((B, L, G, OPG), dtype=np.float32)
    half = L // 2
    for core in range(NCORES):
        b, h = divmod(core, 2)
        l0 = h * half
        # device output is out^T [C=1024 (g*64+o), NR]
        ot = res.results[core]["out"]
        out[b, l0 : l0 + half] = ot.T.reshape(half, G, OPG)
    return out


# revision 35
# speedup vs baseline: 1.1211x; 1.1001x over previous
"""Masked grouped Conv1D (CustomMaskedConv1D) Trainium2 Bass kernel.

Problem (reference semantics):
  inputs    [B=4, L=4096, C=1024] f32
  positions [B=4, L=4096] i32 (sorted)
  kernel    [G=16, OPG=64, IPG=64, K=5] f32
  out[b,l,g,o] = sum_k mask[b,l,k] * sum_i x_pad[b, l+k-2, g*64+i] * W[g,o,i,k]
  mask[b,l,k] = (pos_pad[b, l+k-2] == pos[b,l] + k - 2)

Strategy: data-parallel over (batch x half-sequence) -> 8 shards of 2048 rows
(+2 halo rows each side). Host does lossless layout transforms only
(slicing, zero-pad, transpose, compact weight repack); all dtype casts and
all arithmetic run on device.

Device pipeline per core (DMA-roofline oriented):
  - x + positions read on the SP HWDGE ring (x0 first); weights read as 5
    per-tap compact chunks on the ACT HWDGE ring (both rings feed the same
    16 SDMA engines, so reads/weights/writes flow concurrently)
  - weights shipped compact f32 [128, 40*64] (no block-diag zeros over
    HBM); per-tap strided ACT casts place the diagonal 64x64 blocks into a
    DVE-zeroed bf16 [128, 5120] block-diag lhsT table, in PE consumption
    order (k2 first) so the first matmuls unblock early
  - masks via e-trick: e[j] = pos[j] - j (iota on gpsimd, subtract on DVE,
    all i16 -> DVE 2x mode), then msk_k = is_equal(e[2:], e[4-k:]) as bf16,
    each mask interleaved right before its first production
  - per chunk cc (128 channels = 2 groups): ACT casts x f32->bf16,
    DVE produces ym_k = x * msk_k (bf16, 2x mode), PE runs 20 block-diag
    matmuls (5 taps x 4 n-blocks of 512) accumulating into a [128, 2048]
    psum tile (4 banks), ACT copies psum->SBUF, ACT-ring DMA writes out^T;
    issue order software-pipelines cast(cc+1) ahead of copy(cc)
  - 28 dummy matmuls at kernel start keep the PE HAM clock-gate warm; the
    last chunk drains per-psum-bank (ACT/DVE alternating, sync-ring writes)
    to shorten the kernel tail
Host gathers/transposes shards into [4, 4096, 16, 64].
"""

import os

import numpy as np

import concourse.bass as bass
import concourse.mybir as mybir
import concourse.tile as tile
from concourse import bacc
from concourse.bass_utils import run_bass_kernel_spmd

B, L, C = 4, 4096, 1024
G, OPG, IPG, K = 16, 64, 64, 5
HALO = K // 2  # 2

NCORES = 8
NR = (B * L) // NCORES  # 2048 output rows per core
NP = NR + 2 * HALO  # 2052 padded rows per core
NCC = C // 128  # 8 channel chunks == group pairs
NNB = NR // 512  # 4 n-blocks of 512
NBLK = K * NCC  # 40 weight blocks
TAPS_OFF = (0, 1, 3, 4)  # off-center taps (center tap k=2 has mask==1)
N_WARM = 28  # PE HAM warm-up matmuls

# cache the compiled Bass program + results of the last run
_NC = None
LAST_RESULTS = None


def _build():
    nc = bacc.Bacc(
        "TRN2", target_bir_lowering=False, debug=False, num_devices=NCORES
    )
    bf16 = mybir.dt.bfloat16
    f32 = mybir.dt.float32
    i16 = mybir.dt.int16

    xt_dram = nc.dram_tensor("xt", [C, NP], f32, kind="ExternalInput")
    # positions replicated across 128 partitions on host (layout transform);
    # col j corresponds to padded row m = j - 2, cols [2, NP+2) are real
    ps_dram = nc.dram_tensor("ps", [128, NP + 4], i16, kind="ExternalInput")
    # compact weights [128 ch-in-pair, NBLK*64]: rows 0:64 group (2cc),
    # rows 64:128 group (2cc+1); block j = k*NCC+cc holds W[g,:,:,k].T
    w_dram = nc.dram_tensor("w", [128, NBLK * 64], f32, kind="ExternalInput")
    out_dram = nc.dram_tensor("out", [C, NR], f32, kind="ExternalOutput")

    with tile.TileContext(nc) as tc:
        with (
            tc.tile_pool(name="persist", bufs=1) as pers,
            tc.tile_pool(name="setup", bufs=1) as setup,
            tc.tile_pool(name="stage", bufs=5) as stage,
            tc.tile_pool(name="xbf", bufs=3) as xbf,
            tc.tile_pool(name="ym", bufs=2) as ymp,
            tc.tile_pool(name="osb", bufs=3) as osb,
            tc.tile_pool(name="psum", bufs=2, space="PSUM") as pp,
        ):
            # ---- t0 work on idle engines (no input dependencies) ----
            # iota first: it gates the whole mask chain (e -> msk -> ym)
            iot = pers.tile([128, NP + 4], i16, tag="iota")
            nc.gpsimd.iota(iot[:], pattern=[[1, NP + 4]], base=0,
                           channel_multiplier=0)
            dmy = pers.tile([128, 512], bf16, tag="dmy")
            nc.gpsimd.memset(dmy[:], 0)
            # bf16 block-diag lhsT table: DVE zeros it (DVE is idle until the
            # mask chain starts), ACT casts the diagonal 64x64 blocks in
            w_sb = pers.tile([128, NBLK * 128], bf16, tag="w")
            nc.vector.memset(w_sb[:], 0)

            # ---- reads: SP ring (x0 first: it gates the whole x pipeline) ----
            ps_bc = setup.tile([128, NP + 4], i16, tag="psbc")
            x32s = []
            for cc in range(NCC):
                x32 = stage.tile([128, NP], f32, tag="x32")
                nc.sync.dma_start(x32[:], xt_dram[cc * 128 : (cc + 1) * 128, :])
                x32s.append(x32)
                if cc == 0:
                    nc.sync.dma_start(ps_bc[:], ps_dram[:])

            # ---- weights: 5 per-tap reads on the ACT ring (runs in parallel
            # with the SP-ring reads), dispatched in PE consumption order ----
            w32k = {}
            for k in (2, 0, 1, 3, 4):
                wt = setup.tile([128, NCC * 64], f32, tag=f"w32k{k}")
                nc.scalar.dma_start(wt[:], w_dram[:, k * NCC * 64 : (k + 1) * NCC * 64])
                w32k[k] = wt

            # ---- PE warm-up (keeps HAM clock-gate at 8/8 for real matmuls) ----
            pdum = pp.tile([128, 512], f32, tag="acc", name="pdum")
            for i in range(N_WARM):
                nc.tensor.matmul(pdum[:], dmy[:, 0:128], dmy[:], start=True,
                                 stop=True)

            # ---- masks: e[j] = ps[j] - j, msk_k = (e[2+m] == e[4-k+m]) ----
            # (mask compares are interleaved with cc0's productions below)
            e_t = setup.tile([128, NP + 4], i16, tag="e")
            nc.vector.tensor_tensor(out=e_t[:], in0=ps_bc[:], in1=iot[:],
                                    op=mybir.AluOpType.subtract)
            msk = {}

            # masks stored pairwise ((0,1) and (3,4)) so each chunk's
            # production is 2 double-width DVE ops instead of 4 (halves the
            # per-op fixed overhead); tap k lives at half idx = PAIR_IDX[k]
            PAIRS = ((0, 1), (3, 4))
            PAIR_IDX = {0: 0, 1: 1, 3: 0, 4: 1}
            mskp = {}
            for pa in PAIRS:
                mskp[pa] = pers.tile([128, 2 * NP], bf16, tag=f"msk{pa[0]}",
                                     name=f"mskp{pa[0]}")

            def make_msk(k):
                pa = PAIRS[0] if k in PAIRS[0] else PAIRS[1]
                half = PAIR_IDX[k] * NP
                nc.vector.tensor_tensor(
                    out=mskp[pa][:, half : half + NP],
                    in0=e_t[:, 2 : NP + 2],
                    in1=e_t[:, 4 - k : NP + 4 - k],
                    op=mybir.AluOpType.is_equal,
                )

            def wcast(k):
                # strided casts place tap k's diagonal blocks: partitions 0:64
                # -> block cols [0:64), partitions 64:128 -> [64:128)
                reg = w_sb[:, k * NCC * 128 : (k + 1) * NCC * 128]
                for lo in (0, 64):
                    src = w32k[k][lo : lo + 64, :].rearrange(
                        "p (b c) -> p b c", c=64)
                    dst = reg[lo : lo + 64, :].rearrange(
                        "p (b c) -> p b c", c=128)[:, :, lo : lo + 64]
                    nc.scalar.copy(dst, src)

            def xcast(cc):
                xt = xbf.tile([128, NP], bf16, tag="xt")
                nc.scalar.copy(xt[:], x32s[cc][:])
                return xt

            def prods(cc, xt, first=False):
                # production: masked shifted copies (bf16 full width => 2x),
                # one double-width op per tap pair: [xt|xt] * [msk_a|msk_b].
                # xt is read twice via a broadcast (0-stride) middle dim.
                ym = {}
                xt3 = xt[:].rearrange("p (u n) -> p u n", u=1)
                for pa in PAIRS:
                    if first:
                        for k in pa:
                            make_msk(k)
                    y = ymp.tile([128, 2 * NP], bf16, tag=f"ym{pa[0]}")
                    in1 = mskp[pa][:].rearrange("p (u n) -> p u n", u=2)
                    in0, in1 = bass.broadcast_tensor_aps(xt3, in1)
                    nc.vector.tensor_tensor(
                        out=y[:].rearrange("p (u n) -> p u n", u=2),
                        in0=in0, in1=in1, op=mybir.AluOpType.mult,
                    )
                    for k in pa:
                        ym[k] = y[:, PAIR_IDX[k] * NP : (PAIR_IDX[k] + 1) * NP]
                return ym

            def mms(cc, ptile, xt, ym, taps):
                # conv taps; global tap order is (2, 0, 1, 3, 4): start the
                # psum group on k=2, stop it on k=4
                for ki in taps:
                    wcol = (ki * NCC + cc) * 128
                    lhsT = w_sb[:, wcol : wcol + 128]
                    for nb in range(NNB):
                        n0 = nb * 512
                        if ki == 2:
                            rhs = xt[:, n0 + 2 : n0 + 2 + 512]
                        else:
                            rhs = ym[ki][:, n0 + ki : n0 + ki + 512]
                        nc.tensor.matmul(
                            ptile[:, n0 : n0 + 512], lhsT, rhs,
                            start=(ki == 2), stop=(ki == 4),
                        )

            # ---- software-pipelined schedule over channel chunks ----
            # prologue: cc0 splits its taps around cast(1)/wcast(3,4) so the
            # ACT stream is cast0, wc2/0/1, cast1, wc3/4, cast2, copy0, ...
            xt0 = xcast(0)
            for k in (2, 0, 1):
                wcast(k)
            ym0 = prods(0, xt0, first=True)
            pt0 = pp.tile([128, NR], f32, tag="acc", name="acc0")
            mms(0, pt0, xt0, ym0, (2, 0, 1))
            xt1 = xcast(1)
            for k in (3, 4):
                wcast(k)
            mms(0, pt0, xt0, ym0, (3, 4))

            prev = (pt0, 0)
            xt = xt1
            for cc in range(1, NCC):
                xt_next = xcast(cc + 1) if cc + 1 < NCC else None
                ym = prods(cc, xt)
                ptile = pp.tile([128, NR], f32, tag="acc", name=f"acc{cc}")
                mms(cc, ptile, xt, ym, (2, 0, 1, 3, 4))
                _drain(nc, osb, out_dram, prev)
                prev = (ptile, cc)
                xt = xt_next

            # last chunk drains per-bank to shorten the kernel tail
            _drain(nc, osb, out_dram, prev, fine=True)

    nc.compile()
    return nc


def _drain(nc, osb, out_dram, prev, fine=False):
    ptile, cc = prev
    o_sb = osb.tile([128, NR], mybir.dt.float32, tag="osb")
    rows = out_dram[cc * 128 : (cc + 1) * 128, :]
    if fine:
        # ACT and DVE alternate bank copies (both idle at the tail); all
        # writes dispatch on the idle sync ring so they overlap the copies
        for nb in range(NNB):
            n0 = nb * 512
            if nb % 2 == 0:
                nc.scalar.copy(o_sb[:, n0 : n0 + 512], ptile[:, n0 : n0 + 512])
            else:
                nc.vector.tensor_copy(o_sb[:, n0 : n0 + 512],
                                      ptile[:, n0 : n0 + 512])
            nc.sync.dma_start(rows[:, n0 : n0 + 512], o_sb[:, n0 : n0 + 512])
    else:
        nc.scalar.copy(o_sb[:], ptile[:])
        nc.scalar.dma_start(rows, o_sb[:])


def _get_nc():
    global _NC
    if _NC is None:
        _NC = _build()
    return _NC


def _shard_inputs(inputs, positions, kernel):
    """Host-side lossless layout transforms: slice+pad shards, transpose x,
    compact weight packing. No dtype-lossy changes (positions < 2**15 fit
    int16 exactly)."""
    in_maps = []
    # compact weights [128, NBLK*64] f32: block j = k*NCC + cc
    w_c = np.zeros((128, NBLK * 64), dtype=np.float32)
    for k in range(K):
        for cc in range(NCC):
            j = k * NCC + cc
            w_c[0:64, j * 64 : (j + 1) * 64] = kernel[2 * cc, :, :, k].T
            w_c[64:128, j * 64 : (j + 1) * 64] = kernel[2 * cc + 1, :, :, k].T

    half = L // 2  # 2048
    for core in range(NCORES):
        b, h = divmod(core, 2)
        l0 = h * half
        xs = np.zeros((NP, C), dtype=np.float32)
        ps = np.full((NP + 4,), -1, dtype=np.int16)
        lo, hi = l0 - HALO, l0 + half + HALO
        src_lo, src_hi = max(lo, 0), min(hi, L)
        dst_lo = src_lo - lo
        xs[dst_lo : dst_lo + (src_hi - src_lo)] = inputs[b, src_lo:src_hi]
        ps[2 + dst_lo : 2 + dst_lo + (src_hi - src_lo)] = positions[b, src_lo:src_hi]
        ps_bc = np.ascontiguousarray(np.broadcast_to(ps, (128, NP + 4)))
        xt = np.ascontiguousarray(xs.T)  # [C, NP]
        in_maps.append({"xt": xt, "ps": ps_bc, "w": w_c})
    return in_maps


def kernel(inputs, positions, kernel):
    global LAST_RESULTS
    inputs = np.asarray(inputs, dtype=np.float32)
    positions = np.asarray(positions, dtype=np.int32)
    kernel = np.asarray(kernel, dtype=np.float32)

    nc = _get_nc()
    in_maps = _shard_inputs(inputs, positions, kernel)
    res = run_bass_kernel_spmd(
        nc,
        in_maps,
        core_ids=list(range(NCORES)),
        trace=bool(os.environ.get("BASS_TRACE")),
    )
    LAST_RESULTS = res

    out = np.empty((B, L, G, OPG), dtype=np.float32)
    half = L // 2
    for core in range(NCORES):
        b, h = divmod(core, 2)
        l0 = h * half
        # device output is out^T [C=1024 (g*64+o), NR]
        ot = res.results[core]["out"]
        out[b, l0 : l0 + half] = ot.T.reshape(half, G, OPG)
    return out


# revision 36
# speedup vs baseline: 1.1853x; 1.0572x over previous
"""Masked grouped Conv1D (CustomMaskedConv1D) Trainium2 Bass kernel.

Problem (reference semantics):
  inputs    [B=4, L=4096, C=1024] f32
  positions [B=4, L=4096] i32 (sorted)
  kernel    [G=16, OPG=64, IPG=64, K=5] f32
  out[b,l,g,o] = sum_k mask[b,l,k] * sum_i x_pad[b, l+k-2, g*64+i] * W[g,o,i,k]
  mask[b,l,k] = (pos_pad[b, l+k-2] == pos[b,l] + k - 2)

Strategy: data-parallel over (batch x half-sequence) -> 8 shards of 2048 rows
(+2 halo rows each side). Host does lossless layout transforms only
(slicing, zero-pad, transpose, compact weight repack); all dtype casts and
all arithmetic run on device.

Device pipeline per core (DMA-roofline oriented):
  - x + positions read on the SP HWDGE ring (x0 first); weights read as 5
    per-tap compact chunks on the ACT HWDGE ring (both rings feed the same
    16 SDMA engines, so reads/weights/writes flow concurrently)
  - weights shipped compact f32 [128, 40*64] (no block-diag zeros over
    HBM); per-tap strided ACT casts place the diagonal 64x64 blocks into a
    DVE-zeroed bf16 [128, 5120] block-diag lhsT table, in PE consumption
    order (k2 first) so the first matmuls unblock early
  - masks via e-trick: e[j] = pos[j] - j (iota on gpsimd, subtract on DVE,
    all i16 -> DVE 2x mode), then msk_k = is_equal(e[2:], e[4-k:]) as bf16,
    each mask interleaved right before its first production
  - per chunk cc (128 channels = 2 groups): ACT casts x f32->bf16,
    DVE produces ym_k = x * msk_k (bf16, 2x mode), PE runs 20 block-diag
    matmuls (5 taps x 4 n-blocks of 512) accumulating into a [128, 2048]
    psum tile (4 banks), ACT copies psum->SBUF, ACT-ring DMA writes out^T;
    issue order software-pipelines cast(cc+1) ahead of copy(cc)
  - 28 dummy matmuls at kernel start keep the PE HAM clock-gate warm; the
    last chunk drains per-psum-bank (ACT/DVE alternating, sync-ring writes)
    to shorten the kernel tail
Host gathers/transposes shards into [4, 4096, 16, 64].
"""

import os

import numpy as np

import concourse.bass as bass
import concourse.mybir as mybir
import concourse.tile as tile
from concourse import bacc
from concourse.bass_utils import run_bass_kernel_spmd

B, L, C = 4, 4096, 1024
G, OPG, IPG, K = 16, 64, 64, 5
HALO = K // 2  # 2

NCORES = 8
NR = (B * L) // NCORES  # 2048 output rows per core
NP = NR + 2 * HALO  # 2052 padded rows per core
NCC = C // 128  # 8 channel chunks == group pairs
NNB = NR // 512  # 4 n-blocks of 512
NBLK = K * NCC  # 40 weight blocks
TAPS_OFF = (0, 1, 3, 4)  # off-center taps (center tap k=2 has mask==1)
N_WARM = 28  # PE HAM warm-up matmuls

# cache the compiled Bass program + results of the last run
_NC = None
LAST_RESULTS = None


def _build():
    nc = bacc.Bacc(
        "TRN2", target_bir_lowering=False, debug=False, num_devices=NCORES
    )
    bf16 = mybir.dt.bfloat16
    f32 = mybir.dt.float32
    i16 = mybir.dt.int16

    xt_dram = nc.dram_tensor("xt", [C, NP], f32, kind="ExternalInput")
    # positions replicated across 128 partitions on host (layout transform);
    # col j corresponds to padded row m = j - 2, cols [2, NP+2) are real
    ps_dram = nc.dram_tensor("ps", [128, NP + 4], i16, kind="ExternalInput")
    # compact weights [128 ch-in-pair, NBLK*64]: rows 0:64 group (2cc),
    # rows 64:128 group (2cc+1); block j = k*NCC+cc holds W[g,:,:,k].T
    w_dram = nc.dram_tensor("w", [128, NBLK * 64], f32, kind="ExternalInput")
    out_dram = nc.dram_tensor("out", [C, NR], f32, kind="ExternalOutput")

    with tile.TileContext(nc) as tc:
        with (
            tc.tile_pool(name="persist", bufs=1) as pers,
            tc.tile_pool(name="setup", bufs=1) as setup,
            tc.tile_pool(name="stage", bufs=5) as stage,
            tc.tile_pool(name="xbf", bufs=3) as xbf,
            tc.tile_pool(name="ym", bufs=2) as ymp,
            tc.tile_pool(name="osb", bufs=3) as osb,
            tc.tile_pool(name="psum", bufs=2, space="PSUM") as pp,
        ):
            # ---- t0 work on idle engines (no input dependencies) ----
            # iota first: it gates the whole mask chain (e -> msk -> ym)
            iot = pers.tile([128, NP + 4], i16, tag="iota")
            nc.gpsimd.iota(iot[:], pattern=[[1, NP + 4]], base=0,
                           channel_multiplier=0)
            dmy = pers.tile([128, 512], bf16, tag="dmy")
            nc.gpsimd.memset(dmy[:], 0)
            # bf16 block-diag lhsT table: DVE zeros it (DVE is idle until the
            # mask chain starts), ACT casts the diagonal 64x64 blocks in
            w_sb = pers.tile([128, NBLK * 128], bf16, tag="w")
            nc.vector.memset(w_sb[:], 0)

            # ---- reads: SP ring (x0 first: it gates the whole x pipeline) ----
            ps_bc = setup.tile([128, NP + 4], i16, tag="psbc")
            x32s = []
            for cc in range(NCC):
                x32 = stage.tile([128, NP], f32, tag="x32")
                nc.sync.dma_start(x32[:], xt_dram[cc * 128 : (cc + 1) * 128, :])
                x32s.append(x32)
                if cc == 0:
                    nc.sync.dma_start(ps_bc[:], ps_dram[:])

            # ---- weights: 5 per-tap reads on the ACT ring (runs in parallel
            # with the SP-ring reads), dispatched in PE consumption order ----
            w32k = {}
            for k in (2, 0, 1, 3, 4):
                wt = setup.tile([128, NCC * 64], f32, tag=f"w32k{k}")
                nc.scalar.dma_start(wt[:], w_dram[:, k * NCC * 64 : (k + 1) * NCC * 64])
                w32k[k] = wt

            # ---- PE warm-up (keeps HAM clock-gate at 8/8 for real matmuls) ----
            pdum = pp.tile([128, 512], f32, tag="acc", name="pdum")
            for i in range(N_WARM):
                nc.tensor.matmul(pdum[:], dmy[:, 0:128], dmy[:], start=True,
                                 stop=True)

            # ---- masks: e[j] = ps[j] - j, msk_k = (e[2+m] == e[4-k+m]) ----
            # (mask compares are interleaved with cc0's productions below)
            e_t = setup.tile([128, NP + 4], i16, tag="e")
            nc.vector.tensor_tensor(out=e_t[:], in0=ps_bc[:], in1=iot[:],
                                    op=mybir.AluOpType.subtract)
            msk = {}

            def make_msk(k):
                m = pers.tile([128, NP], bf16, tag=f"msk{k}")
                nc.vector.tensor_tensor(
                    out=m[:], in0=e_t[:, 2 : NP + 2],
                    in1=e_t[:, 4 - k : NP + 4 - k],
                    op=mybir.AluOpType.is_equal,
                )
                msk[k] = m

            def wcast(k):
                # strided casts place tap k's diagonal blocks: partitions 0:64
                # -> block cols [0:64), partitions 64:128 -> [64:128)
                reg = w_sb[:, k * NCC * 128 : (k + 1) * NCC * 128]
                for lo in (0, 64):
                    src = w32k[k][lo : lo + 64, :].rearrange(
                        "p (b c) -> p b c", c=64)
                    dst = reg[lo : lo + 64, :].rearrange(
                        "p (b c) -> p b c", c=128)[:, :, lo : lo + 64]
                    nc.scalar.copy(dst, src)

            def xcast(cc):
                xt = xbf.tile([128, NP], bf16, tag="xt")
                nc.scalar.copy(xt[:], x32s[cc][:])
                return xt

            def prods(cc, xt, first=False):
                # production: masked shifted copies (bf16 full width => 2x);
                # first chunk interleaves each mask compare before its product
                ym = {}
                for k in TAPS_OFF:
                    if first:
                        make_msk(k)
                    y = ymp.tile([128, NP], bf16, tag=f"ym{k}")
                    nc.vector.tensor_tensor(
                        out=y[:], in0=xt[:], in1=msk[k][:],
                        op=mybir.AluOpType.mult,
                    )
                    ym[k] = y
                return ym

            def mms(cc, ptile, xt, ym, taps):
                # conv taps; global tap order is (2, 0, 1, 3, 4): start the
                # psum group on k=2, stop it on k=4
                for ki in taps:
                    wcol = (ki * NCC + cc) * 128
                    lhsT = w_sb[:, wcol : wcol + 128]
                    for nb in range(NNB):
                        n0 = nb * 512
                        if ki == 2:
                            rhs = xt[:, n0 + 2 : n0 + 2 + 512]
                        else:
                            rhs = ym[ki][:, n0 + ki : n0 + ki + 512]
                        nc.tensor.matmul(
                            ptile[:, n0 : n0 + 512], lhsT, rhs,
                            start=(ki == 2), stop=(ki == 4),
                        )

            # ---- software-pipelined schedule over channel chunks ----
            # prologue: cc0 splits its taps around cast(1)/wcast(3,4) so the
            # ACT stream is cast0, wc2/0/1, cast1, wc3/4, cast2, copy0, ...
            xt0 = xcast(0)
            for k in (2, 0, 1):
                wcast(k)
            ym0 = prods(0, xt0, first=True)
            pt0 = pp.tile([128, NR], f32, tag="acc", name="acc0")
            mms(0, pt0, xt0, ym0, (2, 0, 1))
            xt1 = xcast(1)
            for k in (3, 4):
                wcast(k)
            mms(0, pt0, xt0, ym0, (3, 4))

            prev = (pt0, 0)
            xt = xt1
            for cc in range(1, NCC):
                xt_next = xcast(cc + 1) if cc + 1 < NCC else None
                ym = prods(cc, xt)
                ptile = pp.tile([128, NR], f32, tag="acc", name=f"acc{cc}")
                mms(cc, ptile, xt, ym, (2, 0, 1, 3, 4))
                _drain(nc, osb, out_dram, prev)
                prev = (ptile, cc)
                xt = xt_next

            # last chunk drains per-bank to shorten the kernel tail
            _drain(nc, osb, out_dram, prev, fine=True)

    nc.compile()
    return nc


def _drain(nc, osb, out_dram, prev, fine=False):
    ptile, cc = prev
    o_sb = osb.tile([128, NR], mybir.dt.float32, tag="osb")
    rows = out_dram[cc * 128 : (cc + 1) * 128, :]
    if fine:
        # ACT and DVE alternate bank copies (both idle at the tail); all
        # writes dispatch on the idle sync ring so they overlap the copies
        for nb in range(NNB):
            n0 = nb * 512
            if nb % 2 == 0:
                nc.scalar.copy(o_sb[:, n0 : n0 + 512], ptile[:, n0 : n0 + 512])
            else:
                nc.vector.tensor_copy(o_sb[:, n0 : n0 + 512],
                                      ptile[:, n0 : n0 + 512])
            nc.sync.dma_start(rows[:, n0 : n0 + 512], o_sb[:, n0 : n0 + 512])
    else:
        nc.scalar.copy(o_sb[:], ptile[:])
        nc.scalar.dma_start(rows, o_sb[:])


def _get_nc():
    global _NC
    if _NC is None:
        _NC = _build()
    return _NC


def _shard_inputs(inputs, positions, kernel):
    """Host-side lossless layout transforms: slice+pad shards, transpose x,
    compact weight packing. No dtype-lossy changes (positions < 2**15 fit
    int16 exactly)."""
    in_maps = []
    # compact weights [128, NBLK*64] f32: block j = k*NCC + cc
    w_c = np.zeros((128, NBLK * 64), dtype=np.float32)
    for k in range(K):
        for cc in range(NCC):
            j = k * NCC + cc
            w_c[0:64, j * 64 : (j + 1) * 64] = kernel[2 * cc, :, :, k].T
            w_c[64:128, j * 64 : (j + 1) * 64] = kernel[2 * cc + 1, :, :, k].T

    half = L // 2  # 2048
    for core in range(NCORES):
        b, h = divmod(core, 2)
        l0 = h * half
        xs = np.zeros((NP, C), dtype=np.float32)
        ps = np.full((NP + 4,), -1, dtype=np.int16)
        lo, hi = l0 - HALO, l0 + half + HALO
        src_lo, src_hi = max(lo, 0), min(hi, L)
        dst_lo = src_lo - lo
        xs[dst_lo : dst_lo + (src_hi - src_lo)] = inputs[b, src_lo:src_hi]
        ps[2 + dst_lo : 2 + dst_lo + (src_hi - src_lo)] = positions[b, src_lo:src_hi]
        ps_bc = np.ascontiguousarray(np.broadcast_to(ps, (128, NP + 4)))
        xt = np.ascontiguousarray(xs.T)  # [C, NP]
        in_maps.append({"xt": xt, "ps": ps_bc, "w": w_c})
    return in_maps


def kernel(inputs, positions, kernel):
    global LAST_RESULTS
    inputs = np.asarray(inputs, dtype=np.float32)
    positions = np.asarray(positions, dtype=np.int32)
    kernel = np.asarray(kernel, dtype=np.float32)

    nc = _get_nc()
    in_maps = _shard_inputs(inputs, positions, kernel)
    res = run_bass_kernel_spmd(
        nc,
        in_maps,
        core_ids=list(range(NCORES)),
        trace=bool(os.environ.get("BASS_TRACE")),
    )
    LAST_RESULTS = res

    out = np.empty((B, L, G, OPG), dtype=np.float32)
    half = L // 2
    for core in range(NCORES):
        b, h = divmod(core, 2)
        l0 = h * half
        # device output is out^T [C=1024 (g*64+o), NR]
        ot = res.results[core]["out"]
        out[b, l0 : l0 + half] = ot.T.reshape(half, G, OPG)
    return out
